# revision 14
# baseline (speedup 1.0000x reference)
"""CGC MoE routing kernel for Trainium2, 8-core data-parallel over batch.

Problem (per reference):
  B=4096, D_FULL=1024, D_T1=D_T2=512, experts: 4 shared (on x_full),
  4 task-1 (on x_task1), 4 task-2 (on x_task2); each expert is a 2-layer
  ReLU MLP (hidden 512, out 256). Three softmax gates combine expert
  outputs into (out_sh, out1, out2), each [4096, 256] fp32.

Strategy: shard the batch 8 ways (512 rows/core), replicate weights.
Each core computes all 12 experts + gates for its shard; host concats.
Matmuls run in float32r (TF32-like, ~1e-4 rel err) at full PE rate with
fp32 PSUM accumulation.

Layout: activations kept as [feature, batch] ("X'"); host pre-transposes
the x shards so the contraction dim lands on SBUF partitions.
  L1: H[h,b]  = sum_d W1[d,h].T @ X'[d,b]      (W1 tiles stationary)
  L2: EO[b,o] = sum_h H[h,b].T  @ W2[h,o]      (H tiles stationary)
      + ones[1,b].T @ b2[1,o] K=1 matmul for the bias, then ReLU.
  Gates: logits[g,b] = sum_d gW[d,g].T @ X'[d,b]; exp with per-partition
      bias on ACT; PE-transpose to [b,g]; softmax normalization deferred
      (divide by the row sum only at the very end, per output).
  Combine: acc[b,o] (+)= EO_e[b,o] * gate_col[b,1] on DVE
      (scalar_tensor_tensor fused multiply-add), then * 1/sum on ACT.
"""
import sys
import numpy as np

sys.path.insert(0, "/opt/trn_rl_repo")

import concourse.bass as bass
import concourse.mybir as mybir
import concourse.tile as tile
import concourse.masks as masks
from concourse.bass_utils import run_bass_kernel_spmd

F32 = mybir.dt.float32
F32R = mybir.dt.float32r

B = 4096
N_CORES = 8
BC = B // N_CORES          # 512 rows per core
DF, D1, D2 = 1024, 512, 512
HID, OUT = 512, 256
NB = BC // 128             # 4 batch tiles per core
NH = HID // 128            # 4 hidden tiles
E = 4                      # experts per group


def _legalize_waits(nc, max_waits: int = 1):
    """This walrus build supports a single sync wait per instruction;
    hoist extra waits onto standalone single-wait EventSemaphore
    instructions inserted just before (same engine, same order)."""
    uid = 0
    for f in nc.m.functions:
        for blk in f.blocks:
            out = []
            changed = False
            for inst in blk.instructions:
                si = inst.sync_info
                ow = list(si.on_wait) if si and si.on_wait else []
                if len(ow) > max_waits:
                    changed = True
                    for w in ow[:-max_waits]:
                        ev = mybir.InstEventSemaphore(
                            name=f"legalw-{uid}",
                            sync_info=mybir.SyncInfo(on_wait=[w], on_update=[]),
                        )
                        uid += 1
                        ev.engine = inst.engine
                        out.append(ev)
                    inst.sync_info = mybir.SyncInfo(
                        on_wait=ow[-max_waits:],
                        on_update=list(si.on_update) if si.on_update else [],
                    )
                out.append(inst)
            if changed:
                blk.instructions = out
    return nc


def _build_nc():
    nc = bass.Bass()

    def din(name, shape):
        return nc.declare_dram_parameter(name, list(shape), F32, isOutput=False)

    # transposed x shards
    xfT = din("xfT", (DF, BC))
    x1T = din("x1T", (D1, BC))
    x2T = din("x2T", (D2, BC))
    # expert weights
    shW1 = din("shW1", (E, DF, HID)); shB1 = din("shB1", (E, HID))
    shW2 = din("shW2", (E, HID, OUT)); shB2 = din("shB2", (E, OUT))
    t1W1 = din("t1W1", (E, D1, HID)); t1B1 = din("t1B1", (E, HID))
    t1W2 = din("t1W2", (E, HID, OUT)); t1B2 = din("t1B2", (E, OUT))
    t2W1 = din("t2W1", (E, D2, HID)); t2B1 = din("t2B1", (E, HID))
    t2W2 = din("t2W2", (E, HID, OUT)); t2B2 = din("t2B2", (E, OUT))
    # gates
    gshW = din("gshW", (DF, 12)); gshB = din("gshB", (12,))
    g1W = din("g1W", (D1, 8)); g1B = din("g1B", (8,))
    g2W = din("g2W", (D2, 8)); g2B = din("g2B", (8,))

    out_sh = nc.declare_dram_parameter("out_sh", [BC, OUT], F32, isOutput=True)
    out1 = nc.declare_dram_parameter("out1", [BC, OUT], F32, isOutput=True)
    out2 = nc.declare_dram_parameter("out2", [BC, OUT], F32, isOutput=True)

    with tile.TileContext(nc) as tc:
        _emit(nc, tc,
              xfT, x1T, x2T,
              [(t1W1, t1B1, t1W2, t1B2), (t2W1, t2B1, t2W2, t2B2),
               (shW1, shB1, shW2, shB2)],
              [(gshW, gshB, 12), (g1W, g1B, 8), (g2W, g2B, 8)],
              [out_sh, out1, out2])
    _legalize_waits(nc)
    return nc


def _emit(nc, tc, xfT, x1T, x2T, expert_params, gate_params, outs):
    from contextlib import ExitStack
    ctx = ExitStack()
    with ctx:
        xp = ctx.enter_context(tc.tile_pool(name="xp", bufs=1))
        wp = ctx.enter_context(tc.tile_pool(name="wp", bufs=3))
        bp = ctx.enter_context(tc.tile_pool(name="bp", bufs=3))
        hp = ctx.enter_context(tc.tile_pool(name="hp", bufs=2))
        eop = ctx.enter_context(tc.tile_pool(name="eop", bufs=4))
        gp = ctx.enter_context(tc.tile_pool(name="gp", bufs=1))
        accp = ctx.enter_context(tc.tile_pool(name="accp", bufs=1))
        outp = ctx.enter_context(tc.tile_pool(name="outp", bufs=2))
        misc = ctx.enter_context(tc.tile_pool(name="misc", bufs=1))
        # PSUM budget: 8 banks total; each tag gets `bufs` bank-sized slots.
        # ps1 holds {p1, glogit}, ps2 holds {p2, gtr}: 4 tags x 2 = 8 banks.
        ps1 = ctx.enter_context(tc.tile_pool(name="ps1", bufs=2, space="PSUM"))
        ps2 = ctx.enter_context(tc.tile_pool(name="ps2", bufs=2, space="PSUM"))

        # ---- x shards into SBUF as fp32r, packed [128, (d_tile, b)] ----
        def load_xT(dram, d):
            nd = d // 128
            t = xp.tile([128, nd * BC], F32R, tag=f"x_{dram.name}")
            nc.gpsimd.dma_start(
                t[:].rearrange("p (n b) -> p n b", b=BC),
                dram.rearrange("(n p) b -> p n b", p=128))
            return t, nd

        xf, ndf = load_xT(xfT, DF)
        x1, nd1 = load_xT(x1T, D1)
        x2, nd2 = load_xT(x2T, D2)
        xts = [(x1, nd1), (x2, nd2), (xf, ndf)]   # t1, t2, shared order

        # identity for PE transpose; ones row for K=1 bias matmuls
        # (memset/iota are invalid ISA at fp32r -> build fp32, cast-copy)
        ident32 = misc.tile([128, 128], F32, tag="ident32")
        masks.make_identity(nc, ident32[:])
        ident = misc.tile([128, 128], F32R, tag="ident")
        nc.vector.tensor_copy(ident[:], ident32[:])
        ones32 = misc.tile([1, 128], F32, tag="ones32")
        nc.vector.memset(ones32[:], 1.0)
        ones = misc.tile([1, 128], F32R, tag="ones")
        nc.vector.tensor_copy(ones[:], ones32[:])

        # ---- gates ----------------------------------------------------
        # logits [g, b] accumulated over d; exp with per-partition bias;
        # PE-transpose exp'd gates into [b, g] tiles; per-b-tile row sums
        # + reciprocal for the deferred softmax normalization.
        gate_cols = []   # per gate set: ([128, ng] x NB tiles, recip [128,1] x NB)
        for gi, (gW, gB, ng) in enumerate(gate_params):
            xt, nd = {0: (xf, ndf), 1: (x1, nd1), 2: (x2, nd2)}[gi]
            gwt = gp.tile([128, nd * ng], F32R, tag=f"gw{gi}")
            nc.gpsimd.dma_start(
                gwt[:].rearrange("p (n g) -> p n g", g=ng),
                gW.rearrange("(n p) g -> p n g", p=128))
            gbt = gp.tile([ng, 1], F32, tag=f"gb{gi}")
            nc.gpsimd.dma_start(gbt[:], gB.rearrange("(g o) -> g o", o=1))

            lg = ps1.tile([ng, BC], F32, tag="glogit")
            for di in range(nd):
                nc.tensor.matmul(
                    lg[:], gwt[:, di * ng:(di + 1) * ng], xt[:, di * BC:(di + 1) * BC],
                    start=(di == 0), stop=(di == nd - 1))
            eg = gp.tile([ng, BC], F32R, tag=f"eg{gi}")
            nc.scalar.activation(eg[:], lg[:],
                                 mybir.ActivationFunctionType.Exp, bias=gbt[:])
            cols, recips = [], []
            for bi in range(NB):
                pt = ps2.tile([128, ng], F32R, tag="gtr")
                nc.tensor.transpose(pt[:], eg[:, bi * 128:(bi + 1) * 128],
                                    ident[:ng, :ng])
                ct = gp.tile([128, ng], F32, tag=f"gc{gi}_{bi}")
                nc.vector.tensor_copy(ct[:], pt[:])
                st = gp.tile([128, 1], F32, tag=f"gs{gi}_{bi}")
                nc.vector.tensor_reduce(st[:], ct[:], axis=mybir.AxisListType.X,
                                        op=mybir.AluOpType.add)
                rt = gp.tile([128, 1], F32, tag=f"gr{gi}_{bi}")
                nc.vector.reciprocal(rt[:], st[:])
                cols.append(ct)
                recips.append(rt)
            gate_cols.append((cols, recips))

        # accumulator tiles [128, OUT] per output per b-tile
        acc = [[accp.tile([128, OUT], F32, name=f"acc{o}_{bi}", tag=f"acc{o}_{bi}")
                for bi in range(NB)] for o in range(3)]
        acc_init = [[False] * NB for _ in range(3)]

        # expert -> (output index, gate set, gate column) contributions
        # out_sh (gate 0): order [t1(0-3), t2(4-7), sh(8-11)]
        # out1   (gate 1): order [t1(0-3), sh(4-7)]
        # out2   (gate 2): order [t2(0-3), sh(4-7)]
        def contributions(group, e):
            if group == 0:    # t1 expert e
                return [(0, 0, e), (1, 1, e)]
            elif group == 1:  # t2 expert e
                return [(0, 0, 4 + e), (2, 2, e)]
            else:             # shared expert e
                return [(0, 0, 8 + e), (1, 1, 4 + e), (2, 2, 4 + e)]

        # ---- experts --------------------------------------------------
        for group, (W1, B1, W2, B2) in enumerate(expert_params):
            xt, nd = xts[group]
            d_in = nd * 128
            for e in range(E):
                w1 = wp.tile([128, nd * HID], F32R, tag="w1")
                nc.gpsimd.dma_start(
                    w1[:].rearrange("p (n h) -> p n h", h=HID),
                    W1[e].rearrange("(n p) h -> p n h", p=128))
                w2 = wp.tile([128, NH * OUT], F32R, tag="w2")
                nc.gpsimd.dma_start(
                    w2[:].rearrange("p (n o) -> p n o", o=OUT),
                    W2[e].rearrange("(n p) o -> p n o", p=128))
                b1 = bp.tile([128, NH], F32, tag="b1")
                nc.gpsimd.dma_start(
                    b1[:], B1[e].rearrange("(n p) -> p n", p=128))
                b2 = bp.tile([1, OUT], F32R, tag="b2")
                nc.gpsimd.dma_start(b2[:], B2[e].rearrange("(o b) -> o b", o=1))

                # layer 1: H[h, b] tiles packed [128, (hi, b)]
                h = hp.tile([128, NH * BC], F32R, tag="h")
                for hi in range(NH):
                    p1 = ps1.tile([128, BC], F32, tag="p1")
                    for di in range(nd):
                        nc.tensor.matmul(
                            p1[:],
                            w1[:, di * HID + hi * 128: di * HID + (hi + 1) * 128],
                            xt[:, di * BC:(di + 1) * BC],
                            start=(di == 0), stop=(di == nd - 1))
                    nc.scalar.activation(h[:, hi * BC:(hi + 1) * BC], p1[:],
                                         mybir.ActivationFunctionType.Relu,
                                         bias=b1[:, hi:hi + 1])

                # layer 2 + combine, per b-tile
                for bi in range(NB):
                    p2 = ps2.tile([128, OUT], F32, tag="p2")
                    for hi in range(NH):
                        nc.tensor.matmul(
                            p2[:],
                            h[:, hi * BC + bi * 128: hi * BC + (bi + 1) * 128],
                            w2[:, hi * OUT:(hi + 1) * OUT],
                            start=(hi == 0), stop=False)
                    nc.tensor.matmul(p2[:], ones[:], b2[:],
                                     start=False, stop=True)
                    eo = eop.tile([128, OUT], F32, tag="eo")
                    nc.scalar.activation(eo[:], p2[:],
                                         mybir.ActivationFunctionType.Relu)
                    for (o, gs, col) in contributions(group, e):
                        g_ap = gate_cols[gs][0][bi][:, col:col + 1]
                        a = acc[o][bi]
                        if not acc_init[o][bi]:
                            nc.vector.tensor_scalar_mul(a[:], eo[:], g_ap)
                            acc_init[o][bi] = True
                        else:
                            nc.vector.scalar_tensor_tensor(
                                a[:], eo[:], g_ap, a[:],
                                op0=mybir.AluOpType.mult,
                                op1=mybir.AluOpType.add)

        # ---- normalize + store ---------------------------------------
        for o in range(3):
            _, recips = gate_cols[o]
            ot = outp.tile([128, NB * OUT], F32, tag=f"ot{o}")
            for bi in range(NB):
                nc.scalar.mul(ot[:, bi * OUT:(bi + 1) * OUT],
                              acc[o][bi][:], recips[bi][:])
            nc.gpsimd.dma_start(
                outs[o].rearrange("(n p) o -> p n o", p=128),
                ot[:].rearrange("p (n o) -> p n o", o=OUT))


_NC_CACHE = None


def kernel(**inputs):
    global _NC_CACHE
    if _NC_CACHE is None:
        _NC_CACHE = _build_nc()
    nc = _NC_CACHE

    xf = inputs["x_full"]; x1 = inputs["x_task1"]; x2 = inputs["x_task2"]
    shared = {
        "shW1": inputs["sh_W1"], "shB1": inputs["sh_b1"],
        "shW2": inputs["sh_W2"], "shB2": inputs["sh_b2"],
        "t1W1": inputs["t1_W1"], "t1B1": inputs["t1_b1"],
        "t1W2": inputs["t1_W2"], "t1B2": inputs["t1_b2"],
        "t2W1": inputs["t2_W1"], "t2B1": inputs["t2_b1"],
        "t2W2": inputs["t2_W2"], "t2B2": inputs["t2_b2"],
        "gshW": inputs["gsh_W"], "gshB": inputs["gsh_b"],
        "g1W": inputs["g1_W"], "g1B": inputs["g1_b"],
        "g2W": inputs["g2_W"], "g2B": inputs["g2_b"],
    }
    shared = {k: np.ascontiguousarray(v, dtype=np.float32)
              for k, v in shared.items()}

    in_maps = []
    for c in range(N_CORES):
        rows = slice(c * BC, (c + 1) * BC)
        m = dict(shared)
        m["xfT"] = np.ascontiguousarray(xf[rows].T)
        m["x1T"] = np.ascontiguousarray(x1[rows].T)
        m["x2T"] = np.ascontiguousarray(x2[rows].T)
        in_maps.append(m)

    res = run_bass_kernel_spmd(nc, in_maps, list(range(N_CORES)))
    out_sh = np.concatenate([res.results[c]["out_sh"] for c in range(N_CORES)])
    out1 = np.concatenate([res.results[c]["out1"] for c in range(N_CORES)])
    out2 = np.concatenate([res.results[c]["out2"] for c in range(N_CORES)])
    return (out_sh, out1, out2)


# revision 17
# speedup vs baseline: 1.0561x; 1.0561x over previous
"""CGC MoE routing kernel for Trainium2, 8-core data-parallel over batch.

Problem (per reference):
  B=4096, D_FULL=1024, D_T1=D_T2=512, experts: 4 shared (on x_full),
  4 task-1 (on x_task1), 4 task-2 (on x_task2); each expert is a 2-layer
  ReLU MLP (hidden 512, out 256). Three softmax gates combine expert
  outputs into (out_sh, out1, out2), each [4096, 256] fp32.

Strategy: shard the batch 8 ways (512 rows/core), replicate weights.
Each core computes all 12 experts + gates for its shard; host concats.
Matmuls run in float32r (TF32-like, ~2e-4 rel err) with fp32 PSUM
accumulation. Matmul operands are declared float32r in DRAM (same bytes
as fp32) so plain HWDGE DMAs feed them with no cast step.

Layout: activations kept as [feature, batch]; the host pre-transposes
the x shards and pre-packs weights into SBUF partition layout so every
DMA is a long contiguous run per partition.
  L1: H[h,b]  = sum_d W1[d,h].T @ X'[d,b]      (W1 tiles stationary)
  L2: EO[b,o] = sum_h H[h,b].T  @ W2[h,o]      (H tiles stationary)
      + ones[1,b].T @ b2[1,o] K=1 matmul for the bias, then ReLU.
  Gates: logits[g,b] = sum_d gW[d,g].T @ X'[d,b]; exp with per-partition
      bias on ACT; PE-transpose to [b,g]; columns pre-scaled by 1/rowsum
      so the combine uses normalized gates directly.
  Combine: acc[b,o] (+)= EO_e[b,o] * gate_col[b,1] on DVE
      (scalar_tensor_tensor fused multiply-add); acc DMAs straight out.
"""
import sys
import numpy as np

sys.path.insert(0, "/opt/trn_rl_repo")

import concourse.bass as bass
import concourse.mybir as mybir
import concourse.tile as tile
import concourse.masks as masks
from concourse.bass_utils import run_bass_kernel_spmd

F32 = mybir.dt.float32
F32R = mybir.dt.float32r

B = 4096
N_CORES = 8
BC = B // N_CORES          # 512 rows per core
DF, D1, D2 = 1024, 512, 512
HID, OUT = 512, 256
NB = BC // 128             # 4 batch tiles per core
NH = HID // 128            # 4 hidden tiles
E = 4                      # experts per group


def _legalize_waits(nc, max_waits: int = 1):
    """This walrus build supports a single sync wait per instruction;
    hoist extra waits onto standalone single-wait EventSemaphore
    instructions inserted just before (same engine, same order)."""
    uid = 0
    for f in nc.m.functions:
        for blk in f.blocks:
            out = []
            changed = False
            for inst in blk.instructions:
                si = inst.sync_info
                ow = list(si.on_wait) if si and si.on_wait else []
                if len(ow) > max_waits:
                    changed = True
                    for w in ow[:-max_waits]:
                        ev = mybir.InstEventSemaphore(
                            name=f"legalw-{uid}",
                            sync_info=mybir.SyncInfo(on_wait=[w], on_update=[]),
                        )
                        uid += 1
                        ev.engine = inst.engine
                        out.append(ev)
                    inst.sync_info = mybir.SyncInfo(
                        on_wait=ow[-max_waits:],
                        on_update=list(si.on_update) if si.on_update else [],
                    )
                out.append(inst)
            if changed:
                blk.instructions = out
    return nc


def _build_nc():
    nc = bass.Bass()

    def din(name, shape, dt=F32R):
        return nc.declare_dram_parameter(name, list(shape), dt, isOutput=False)

    # transposed x shards
    xfT = din("xfT", (DF, BC))
    x1T = din("x1T", (D1, BC))
    x2T = din("x2T", (D2, BC))
    # host-packed expert weights: W1 [E,128,nd*HID], W2 [E,128,NH*OUT],
    # b1 [E,128,NH] (fp32), b2 [E,OUT]
    t1W1 = din("t1W1", (E, 128, (D1 // 128) * HID))
    t2W1 = din("t2W1", (E, 128, (D2 // 128) * HID))
    shW1 = din("shW1", (E, 128, (DF // 128) * HID))
    t1W2 = din("t1W2", (E, 128, NH * OUT))
    t2W2 = din("t2W2", (E, 128, NH * OUT))
    shW2 = din("shW2", (E, 128, NH * OUT))
    t1B1 = din("t1B1", (E, 128, NH), F32)
    t2B1 = din("t2B1", (E, 128, NH), F32)
    shB1 = din("shB1", (E, 128, NH), F32)
    t1B2 = din("t1B2", (E, OUT))
    t2B2 = din("t2B2", (E, OUT))
    shB2 = din("shB2", (E, OUT))
    # host-packed gate weights [128, nd*ng]; biases [ng]
    gshW = din("gshW", (128, (DF // 128) * 12))
    g1W = din("g1W", (128, (D1 // 128) * 8))
    g2W = din("g2W", (128, (D2 // 128) * 8))
    gshB = din("gshB", (12,), F32)
    g1B = din("g1B", (8,), F32)
    g2B = din("g2B", (8,), F32)

    out_sh = nc.declare_dram_parameter("out_sh", [BC, OUT], F32, isOutput=True)
    out1 = nc.declare_dram_parameter("out1", [BC, OUT], F32, isOutput=True)
    out2 = nc.declare_dram_parameter("out2", [BC, OUT], F32, isOutput=True)

    with tile.TileContext(nc) as tc:
        _emit(nc, tc,
              {"xf": xfT, "x1": x1T, "x2": x2T},
              # expert groups in processing order: t1, sh, t2
              [("t1", t1W1, t1B1, t1W2, t1B2),
               ("sh", shW1, shB1, shW2, shB2),
               ("t2", t2W1, t2B1, t2W2, t2B2)],
              [(gshW, gshB, 12), (g1W, g1B, 8), (g2W, g2B, 8)],
              [out_sh, out1, out2])
    _legalize_waits(nc)
    return nc


def _emit(nc, tc, xins, expert_groups, gate_params, outs):
    from contextlib import ExitStack
    ctx = ExitStack()
    with ctx:
        xp = ctx.enter_context(tc.tile_pool(name="xp", bufs=1))
        wp = ctx.enter_context(tc.tile_pool(name="wp", bufs=2))
        bp = ctx.enter_context(tc.tile_pool(name="bp", bufs=3))
        hp = ctx.enter_context(tc.tile_pool(name="hp", bufs=2))
        eop = ctx.enter_context(tc.tile_pool(name="eop", bufs=4))
        gp = ctx.enter_context(tc.tile_pool(name="gp", bufs=1))
        accp = ctx.enter_context(tc.tile_pool(name="accp", bufs=1))
        misc = ctx.enter_context(tc.tile_pool(name="misc", bufs=1))
        # PSUM: 8 banks. p1 gets 3 (L1 + gate logits share the tag),
        # p2 gets 5 (L2 + gate transposes share the tag).
        ps1 = ctx.enter_context(tc.tile_pool(name="ps1", bufs=3, space="PSUM"))
        ps2 = ctx.enter_context(tc.tile_pool(name="ps2", bufs=5, space="PSUM"))

        # ---- x shards: one [128, BC] fp32r tile per 128-feature slice --
        def load_xT(key):
            dram = xins[key]
            nd = dram.shape[0] // 128
            ts = []
            for di in range(nd):
                t = xp.tile([128, BC], F32R, name=f"x_{key}_{di}",
                            tag=f"x_{key}_{di}")
                nc.sync.dma_start(t[:], dram[di * 128:(di + 1) * 128, :])
                ts.append(t)
            return ts

        x1 = load_xT("x1")   # first expert group (t1) runs first
        xf = load_xT("xf")
        x2 = load_xT("x2")
        xts = {"t1": x1, "sh": xf, "t2": x2}

        # identity for PE transpose; ones row for K=1 bias matmuls
        # (memset/iota are invalid ISA at fp32r -> build fp32, cast-copy)
        ident32 = misc.tile([128, 128], F32, tag="ident32")
        masks.make_identity(nc, ident32[:])
        ident = misc.tile([128, 128], F32R, tag="ident")
        nc.vector.tensor_copy(ident[:], ident32[:])
        ones32 = misc.tile([1, 128], F32, tag="ones32")
        nc.vector.memset(ones32[:], 1.0)
        ones = misc.tile([1, 128], F32R, tag="ones")
        nc.vector.tensor_copy(ones[:], ones32[:])

        # accumulator tiles [128, OUT] per output per b-tile
        acc = [[accp.tile([128, OUT], F32, name=f"acc{o}_{bi}",
                          tag=f"acc{o}_{bi}")
                for bi in range(NB)] for o in range(3)]
        acc_init = [[False] * NB for _ in range(3)]

        # ---- expert bodies ---------------------------------------------
        def emit_expert_l1(group, W1, B1, W2, B2, e):
            xt = xts[group]
            nd = len(xt)
            w1 = wp.tile([128, nd * HID], F32R, name="w1", tag="w1")
            nc.sync.dma_start(w1[:], W1[e])
            w2 = wp.tile([128, NH * OUT], F32R, name="w2", tag="w2")
            nc.sync.dma_start(w2[:], W2[e])
            b1 = bp.tile([128, NH], F32, name="b1", tag="b1")
            nc.sync.dma_start(b1[:], B1[e])
            b2 = bp.tile([1, OUT], F32R, name="b2", tag="b2")
            nc.sync.dma_start(b2[:], B2[e].rearrange("(a o) -> a o", a=1))

            # layer 1: H[h, b] tiles packed [128, (hi, b)]
            h = hp.tile([128, NH * BC], F32R, name="h", tag="h")
            for hi in range(NH):
                p1 = ps1.tile([128, BC], F32, name="p1", tag="p1")
                for di in range(nd):
                    nc.tensor.matmul(
                        p1[:],
                        w1[:, di * HID + hi * 128: di * HID + (hi + 1) * 128],
                        xt[di][:],
                        start=(di == 0), stop=(di == nd - 1))
                nc.scalar.activation(h[:, hi * BC:(hi + 1) * BC], p1[:],
                                     mybir.ActivationFunctionType.Relu,
                                     bias=b1[:, hi:hi + 1])
            return h, w2, b2

        def emit_expert_l2(group, e, h, w2, b2, gate_cols):
            for bi in range(NB):
                p2 = ps2.tile([128, OUT], F32, name="p2", tag="p2")
                for hi in range(NH):
                    nc.tensor.matmul(
                        p2[:],
                        h[:, hi * BC + bi * 128: hi * BC + (bi + 1) * 128],
                        w2[:, hi * OUT:(hi + 1) * OUT],
                        start=(hi == 0), stop=False)
                nc.tensor.matmul(p2[:], ones[:], b2[:], start=False, stop=True)
                eo = eop.tile([128, OUT], F32, name="eo", tag="eo")
                nc.scalar.activation(eo[:], p2[:],
                                     mybir.ActivationFunctionType.Relu)
                for (o, gs, col) in contributions(group, e):
                    g_ap = gate_cols[gs][bi][:, col:col + 1]
                    a = acc[o][bi]
                    if not acc_init[o][bi]:
                        nc.vector.tensor_scalar_mul(a[:], eo[:], g_ap)
                        acc_init[o][bi] = True
                    else:
                        nc.vector.scalar_tensor_tensor(
                            a[:], eo[:], g_ap, a[:],
                            op0=mybir.AluOpType.mult,
                            op1=mybir.AluOpType.add)
                    if _is_last_contrib(group, e, o):
                        nc.sync.dma_start(
                            outs[o][bi * 128:(bi + 1) * 128, :], a[:])

        # expert -> (output index, gate set, gate column) contributions
        # gates: gsh over [t1(0-3), t2(4-7), sh(8-11)]
        #        g1  over [t1(0-3), sh(4-7)]; g2 over [t2(0-3), sh(4-7)]
        def contributions(group, e):
            if group == "t1":
                return [(0, 0, e), (1, 1, e)]
            elif group == "t2":
                return [(0, 0, 4 + e), (2, 2, e)]
            else:
                return [(0, 0, 8 + e), (1, 1, 4 + e), (2, 2, 4 + e)]

        # processing order: t1(0..3), sh(0..3), t2(0..3)
        # out1 finishes at sh_3 (its DMA overlaps the t2 group);
        # out_sh/out2 finish at t2_3.
        def _is_last_contrib(group, e, o):
            if o == 1:
                return group == "sh" and e == E - 1
            return group == "t2" and e == E - 1

        # ---- gates ------------------------------------------------------
        def emit_gates():
            gate_cols = []
            for gi, (gW, gB, ng) in enumerate(gate_params):
                xt = {0: xf, 1: x1, 2: x2}[gi]
                nd = len(xt)
                gwt = gp.tile([128, nd * ng], F32R, name=f"gw{gi}",
                              tag=f"gw{gi}")
                nc.sync.dma_start(gwt[:], gW[:])
                gbt = gp.tile([ng, 1], F32, name=f"gb{gi}", tag=f"gb{gi}")
                nc.sync.dma_start(gbt[:], gB.rearrange("(g o) -> g o", o=1))

                lg = ps1.tile([ng, BC], F32, name="lg", tag="p1")
                for di in range(nd):
                    nc.tensor.matmul(
                        lg[:], gwt[:, di * ng:(di + 1) * ng], xt[di][:],
                        start=(di == 0), stop=(di == nd - 1))
                eg = gp.tile([ng, BC], F32R, name=f"eg{gi}", tag=f"eg{gi}")
                nc.scalar.activation(eg[:], lg[:],
                                     mybir.ActivationFunctionType.Exp,
                                     bias=gbt[:])
                cols = []
                for bi in range(NB):
                    pt = ps2.tile([128, ng], F32R, name="gtr", tag="p2")
                    nc.tensor.transpose(pt[:], eg[:, bi * 128:(bi + 1) * 128],
                                        ident[:ng, :ng])
                    ct = gp.tile([128, ng], F32, name=f"gc{gi}_{bi}",
                                 tag=f"gc{gi}_{bi}")
                    nc.vector.tensor_copy(ct[:], pt[:])
                    st = gp.tile([128, 1], F32, name=f"gs{gi}_{bi}",
                                 tag=f"gs{gi}_{bi}")
                    nc.vector.tensor_reduce(st[:], ct[:],
                                            axis=mybir.AxisListType.X,
                                            op=mybir.AluOpType.add)
                    rt = gp.tile([128, 1], F32, name=f"gr{gi}_{bi}",
                                 tag=f"gr{gi}_{bi}")
                    nc.vector.reciprocal(rt[:], st[:])
                    # pre-scale the gate columns: combine uses these directly
                    nc.vector.tensor_scalar_mul(ct[:], ct[:], rt[:])
                    cols.append(ct)
                gate_cols.append(cols)
            return gate_cols

        # ---- emission order --------------------------------------------
        # expert t1_0's weights + L1 first so PE starts as soon as the
        # first x/w DMAs land; gates next (only the DVE combine waits on
        # them); then expert t1_0's L2+combine and the remaining experts.
        first = True
        gate_cols = None
        for group, W1, B1, W2, B2 in expert_groups:
            for e in range(E):
                l1 = emit_expert_l1(group, W1, B1, W2, B2, e)
                if first:
                    gate_cols = emit_gates()
                    first = False
                emit_expert_l2(group, e, *l1, gate_cols)


_NC_CACHE = None


def _pack_inputs(inputs):
    """Host-side packing into SBUF partition layouts (pure relayout)."""
    def pack_w(w, inner):     # [E, D, inner] -> [E, 128, (D/128)*inner]
        e, dd, nn = w.shape
        nd = dd // 128
        return np.ascontiguousarray(
            w.reshape(e, nd, 128, nn).transpose(0, 2, 1, 3).reshape(e, 128, nd * nn),
            dtype=np.float32)

    def pack_b1(b):           # [E, HID] -> [E, 128, NH] with [p, n] = b[n*128+p]
        e, hh = b.shape
        nh = hh // 128
        return np.ascontiguousarray(
            b.reshape(e, nh, 128).transpose(0, 2, 1), dtype=np.float32)

    def pack_gw(w):           # [D, ng] -> [128, (D/128)*ng]
        dd, ng = w.shape
        nd = dd // 128
        return np.ascontiguousarray(
            w.reshape(nd, 128, ng).transpose(1, 0, 2).reshape(128, nd * ng),
            dtype=np.float32)

    f32 = lambda a: np.ascontiguousarray(a, dtype=np.float32)
    return {
        "t1W1": pack_w(inputs["t1_W1"], HID), "t1B1": pack_b1(inputs["t1_b1"]),
        "t1W2": pack_w(inputs["t1_W2"], OUT), "t1B2": f32(inputs["t1_b2"]),
        "t2W1": pack_w(inputs["t2_W1"], HID), "t2B1": pack_b1(inputs["t2_b1"]),
        "t2W2": pack_w(inputs["t2_W2"], OUT), "t2B2": f32(inputs["t2_b2"]),
        "shW1": pack_w(inputs["sh_W1"], HID), "shB1": pack_b1(inputs["sh_b1"]),
        "shW2": pack_w(inputs["sh_W2"], OUT), "shB2": f32(inputs["sh_b2"]),
        "gshW": pack_gw(inputs["gsh_W"]), "gshB": f32(inputs["gsh_b"]),
        "g1W": pack_gw(inputs["g1_W"]), "g1B": f32(inputs["g1_b"]),
        "g2W": pack_gw(inputs["g2_W"]), "g2B": f32(inputs["g2_b"]),
    }


def kernel(**inputs):
    global _NC_CACHE
    if _NC_CACHE is None:
        _NC_CACHE = _build_nc()
    nc = _NC_CACHE

    shared = _pack_inputs(inputs)
    xf, x1, x2 = inputs["x_full"], inputs["x_task1"], inputs["x_task2"]

    in_maps = []
    for c in range(N_CORES):
        rows = slice(c * BC, (c + 1) * BC)
        m = dict(shared)
        m["xfT"] = np.ascontiguousarray(xf[rows].T)
        m["x1T"] = np.ascontiguousarray(x1[rows].T)
        m["x2T"] = np.ascontiguousarray(x2[rows].T)
        in_maps.append(m)

    res = run_bass_kernel_spmd(nc, in_maps, list(range(N_CORES)))
    out_sh = np.concatenate([res.results[c]["out_sh"] for c in range(N_CORES)])
    out1 = np.concatenate([res.results[c]["out1"] for c in range(N_CORES)])
    out2 = np.concatenate([res.results[c]["out2"] for c in range(N_CORES)])
    return (out_sh, out1, out2)
